# revision 9
# baseline (speedup 1.0000x reference)
"""MinGRU (B=4, T=4096, D=1024) Trainium2 kernel, 8-core SPMD.

Sharding: core i handles (batch b = i//2, output-channel half j = i%2).
Each core computes u_z = x[b] @ Wz[half].T, u_h = x[b] @ Wh[half].T,
z = sigmoid(u_z + bz), a = 1 - z, bvec = z * (u_h + bh), then the
recurrence h_t = a_t * h_{t-1} + b_t via the hardware tensor_tensor_scan.

Matmuls are fp32r (FP22 self-loading; no separate LDWEIGHTS, so the PE
stream is gap-free). x lives fully resident in SBUF (16 MiB = 128 KB per
partition, loaded once at startup), so the steady-state loop does no
input DMA at all and the only per-iteration DMA is the bf16 h output
(4 MiB). Epilogue keeps one PSUM reader per bank: ACT does
z = sigmoid(pz + bz); DVE does a = 1 - z, b = (ph + bh) * z, and the
scan (fp32 state, bf16 out). Host pre-transposes x and W so every DMA is
row-contiguous and converts h back to fp32.

The sustained per-iteration time is pinned by the PE clock governor
(K=4/8 under sustained 8-core load): ~159 us = 262144 streamed columns
at ~1.65 GHz. This version minimizes everything above that floor and
keeps the PE stream dense enough to catch the full-clock state when the
governor allows.
"""

import numpy as np

_B, _T, _D = 4, 4096, 1024
_EH = 512
_NG = _EH // 128
_TT = 512
_NT = _T // _TT    # 8 resident t-tiles
_NK = _D // 128


def _build(reps=1, loop_n=None):
    from contextlib import ExitStack
    from concourse import bacc, mybir, tile

    f32 = mybir.dt.float32
    f32r = mybir.dt.float32r
    bf16 = mybir.dt.bfloat16
    AF = mybir.ActivationFunctionType
    OP = mybir.AluOpType

    nc = bacc.Bacc("TRN2", debug=False, num_devices=8)
    xt = nc.dram_tensor("xt", [_D, _T], f32, kind="ExternalInput").ap()
    wzt = nc.dram_tensor("wzt", [_D, _EH], f32, kind="ExternalInput").ap()
    wht = nc.dram_tensor("wht", [_D, _EH], f32, kind="ExternalInput").ap()
    bzt = nc.dram_tensor("bzt", [128, _NG], f32, kind="ExternalInput").ap()
    bht = nc.dram_tensor("bht", [128, _NG], f32, kind="ExternalInput").ap()
    hout = nc.dram_tensor("h", [_EH, _T], bf16, kind="ExternalOutput").ap()

    with tile.TileContext(nc) as tc, ExitStack() as ctx:
        wpool = ctx.enter_context(tc.tile_pool(name="w", bufs=1))
        vpool = ctx.enter_context(tc.tile_pool(name="v", bufs=3))
        hpool = ctx.enter_context(tc.tile_pool(name="h", bufs=2))
        ppool = ctx.enter_context(tc.tile_pool(name="p", bufs=3, space="PSUM"))
        plast = ctx.enter_context(tc.tile_pool(name="pl", bufs=1, space="PSUM"))

        # x fully resident: 8 t-tiles of [128, (k tt)] fp32r = 128 KB/partition.
        xres = []
        for t in range(_NT):
            xres_t = wpool.tile([128, _NK * _TT], f32r, tag=f"x{t}")
            xres.append(xres_t)
        wz_sb = wpool.tile([128, _NK * _EH], f32r, tag="wz")
        wh_sb = wpool.tile([128, _NK * _EH], f32r, tag="wh")
        bz_sb = wpool.tile([128, _NG], f32, tag="bz")
        bh_sb = wpool.tile([128, _NG], f32, tag="bh")

        def x_chunk(t, ks, nk):
            nc.sync.dma_start(
                xres[t][:, ks * _TT:(ks + nk) * _TT].rearrange(
                    "p (k t) -> p k t", k=nk),
                xt.rearrange("(k p) t -> p k t", p=128)[
                    :, ks:ks + nk, t * _TT:(t + 1) * _TT].bitcast(f32r),
            )

        def w_chunk(k):
            nc.sync.dma_start(
                wz_sb[:, k * _EH:(k + 1) * _EH],
                wzt[k * 128:(k + 1) * 128, :].bitcast(f32r))
            nc.sync.dma_start(
                wh_sb[:, k * _EH:(k + 1) * _EH],
                wht[k * 128:(k + 1) * 128, :].bitcast(f32r))

        x_chunk(0, 0, 4)
        w_chunk(0)
        w_chunk(1)
        nc.sync.dma_start(bz_sb[:], bzt)
        nc.sync.dma_start(bh_sb[:], bht)
        x_chunk(0, 4, 4)
        for k in range(2, _NK):
            w_chunk(k)
        for t in range(1, _NT):
            x_chunk(t, 0, 4)
            x_chunk(t, 4, 4)

        def body(first):
          hprev = [None] * _NG
          for t in range(_NT):
            xs = xres[t]
            for g in range(_NG):
                last = (t == _NT - 1 and g == _NG - 1)
                halves = ((0, _TT // 2), (_TT // 2, _TT // 2)) if last \
                    else ((0, _TT),)
                prev_ap = None if t == 0 else hprev[g][:, _TT - 1:_TT]
                for (c0, w) in halves:
                    pool = plast if last else ppool
                    pz = pool.tile([128, w], f32, tag="pzl" if last else "pz")
                    ph = pool.tile([128, w], f32, tag="phl" if last else "ph")
                    for k in range(_NK):
                        nc.tensor.matmul(
                            pz[:],
                            lhsT=wz_sb[:, k * _EH + g * 128:
                                       k * _EH + (g + 1) * 128],
                            rhs=xs[:, k * _TT + c0: k * _TT + c0 + w],
                            start=(k == 0),
                            stop=(k == _NK - 1),
                        )
                    for k in range(_NK):
                        nc.tensor.matmul(
                            ph[:],
                            lhsT=wh_sb[:, k * _EH + g * 128:
                                       k * _EH + (g + 1) * 128],
                            rhs=xs[:, k * _TT + c0: k * _TT + c0 + w],
                            start=(k == 0),
                            stop=(k == _NK - 1),
                        )
                    z = vpool.tile([128, w], f32, tag="z")
                    nc.scalar.activation(z[:], pz[:], AF.Sigmoid,
                                         bias=bz_sb[:, g:g + 1])
                    av = vpool.tile([128, w], f32, tag="a")
                    nc.vector.tensor_scalar(
                        av[:], z[:], -1.0, 1.0, OP.mult, OP.add)
                    bv = vpool.tile([128, w], f32, tag="b")
                    nc.vector.scalar_tensor_tensor(
                        bv[:], ph[:], bh_sb[:, g:g + 1], z[:],
                        OP.add, OP.mult)
                    hb = hpool.tile([128, w], bf16, tag=f"h{g}")
                    init = 0.0 if prev_ap is None else prev_ap
                    nc.vector.tensor_tensor_scan(hb[:], av[:], bv[:], init,
                                                 OP.mult, OP.add)
                    prev_ap = hb[:, w - 1:w]
                    if not last:
                        hprev[g] = hb
                    nc.sync.dma_start(
                        hout[g * 128:(g + 1) * 128,
                             t * _TT + c0: t * _TT + c0 + w], hb[:])

        if loop_n is not None:
            body(True)
            from concourse import mybir as _mb
            # 2x unrolled: halves the loop reset/branch overhead per body.
            # Total bodies = 1 + 2*(loop_n//2) = loop_n + 1, matching the
            # slope bench's divisor.
            assert loop_n % 2 == 0
            with tc.For_i(0, loop_n // 2, 1, hint_engines=(
                    _mb.EngineType.PE, _mb.EngineType.SP,
                    _mb.EngineType.DVE, _mb.EngineType.Activation),
                    staggered_reset=True):
                body(False)
                body(False)
        else:
            for rep in range(reps):
                body(rep == 0)
    nc.compile()
    return nc


_NC_CACHE = None


def _shard_inputs(inputs):
    x = np.asarray(inputs["x"], dtype=np.float32)
    Wz = np.asarray(inputs["Wz"], dtype=np.float32)
    bz = np.asarray(inputs["bz"], dtype=np.float32)
    Wh = np.asarray(inputs["Wh"], dtype=np.float32)
    bh = np.asarray(inputs["bh"], dtype=np.float32)

    wzT = np.ascontiguousarray(Wz.T)
    whT = np.ascontiguousarray(Wh.T)

    in_maps = []
    for i in range(8):
        b, j = i // 2, i % 2
        sl = slice(j * _EH, (j + 1) * _EH)
        in_maps.append({
            "xt": np.ascontiguousarray(x[b].T),
            "wzt": np.ascontiguousarray(wzT[:, sl]),
            "wht": np.ascontiguousarray(whT[:, sl]),
            "bzt": np.ascontiguousarray(bz[sl].reshape(_NG, 128).T),
            "bht": np.ascontiguousarray(bh[sl].reshape(_NG, 128).T),
        })
    return in_maps


def run(inputs, trace=False, tmpdir=None):
    global _NC_CACHE
    from concourse.bass_utils import run_bass_kernel_spmd

    if _NC_CACHE is None:
        _NC_CACHE = _build()
    nc = _NC_CACHE
    in_maps = _shard_inputs(inputs)
    res = run_bass_kernel_spmd(
        nc, in_maps, core_ids=list(range(8)), trace=trace, tmpdir=tmpdir)
    out = np.empty((_B, _T, _D), dtype=np.float32)
    for i in range(8):
        b, j = i // 2, i % 2
        out[b, :, j * _EH:(j + 1) * _EH] = res.results[i]["h"].astype(np.float32).T
    return out, res


def kernel(**inputs):
    out, _ = run(inputs, trace=False)
    return out


# revision 10
# speedup vs baseline: 1.0310x; 1.0310x over previous
"""MinGRU (B=4, T=4096, D=1024) Trainium2 kernel, 8-core SPMD.

Sharding: core i handles (batch b = i//2, output-channel half j = i%2).
Each core computes u_z = x[b] @ Wz[half].T, u_h = x[b] @ Wh[half].T,
z = sigmoid(u_z + bz), a = 1 - z, bvec = z * (u_h + bh), then the
recurrence h_t = a_t * h_{t-1} + b_t via the hardware tensor_tensor_scan.

Matmuls are fp32r (FP22 self-loading; no separate LDWEIGHTS, so the PE
stream is gap-free). x lives fully resident in SBUF (16 MiB = 128 KB per
partition, loaded once at startup), so the steady-state loop does no
input DMA at all and the only per-iteration DMA is the bf16 h output
(4 MiB). Epilogue keeps one PSUM reader per bank: ACT does
z = sigmoid(pz + bz); DVE does a = 1 - z, b = (ph + bh) * z, and the
scan (fp32 state, bf16 out). Host pre-transposes x and W so every DMA is
row-contiguous and converts h back to fp32.

The sustained per-iteration time is pinned by the PE clock governor
(K=4/8 under sustained 8-core load): ~159 us = 262144 streamed columns
at ~1.65 GHz. This version minimizes everything above that floor and
keeps the PE stream dense enough to catch the full-clock state when the
governor allows.
"""

import numpy as np

_B, _T, _D = 4, 4096, 1024
_EH = 512
_NG = _EH // 128
_TT = 512
_NT = _T // _TT    # 8 resident t-tiles
_NK = _D // 128


def _build(reps=1, loop_n=None):
    from contextlib import ExitStack
    from concourse import bacc, mybir, tile

    f32 = mybir.dt.float32
    f32r = mybir.dt.float32r
    bf16 = mybir.dt.bfloat16
    AF = mybir.ActivationFunctionType
    OP = mybir.AluOpType

    nc = bacc.Bacc("TRN2", debug=False, num_devices=8)
    xt = nc.dram_tensor("xt", [_D, _T], f32, kind="ExternalInput").ap()
    wzt = nc.dram_tensor("wzt", [_D, _EH], f32, kind="ExternalInput").ap()
    wht = nc.dram_tensor("wht", [_D, _EH], f32, kind="ExternalInput").ap()
    bzt = nc.dram_tensor("bzt", [128, _NG], f32, kind="ExternalInput").ap()
    bht = nc.dram_tensor("bht", [128, _NG], f32, kind="ExternalInput").ap()
    hout = nc.dram_tensor("h", [_EH, _T], bf16, kind="ExternalOutput").ap()

    with tile.TileContext(nc) as tc, ExitStack() as ctx:
        wpool = ctx.enter_context(tc.tile_pool(name="w", bufs=1))
        vpool = ctx.enter_context(tc.tile_pool(name="v", bufs=3))
        hpool = ctx.enter_context(tc.tile_pool(name="h", bufs=2))
        ppool = ctx.enter_context(tc.tile_pool(name="p", bufs=3, space="PSUM"))
        plast = ctx.enter_context(tc.tile_pool(name="pl", bufs=1, space="PSUM"))

        # x fully resident: 8 t-tiles of [128, (k tt)] fp32r = 128 KB/partition.
        xres = []
        for t in range(_NT):
            xres_t = wpool.tile([128, _NK * _TT], f32r, tag=f"x{t}")
            xres.append(xres_t)
        wz_sb = wpool.tile([128, _NK * _EH], f32r, tag="wz")
        wh_sb = wpool.tile([128, _NK * _EH], f32r, tag="wh")
        bz_sb = wpool.tile([128, _NG], f32, tag="bz")
        bh_sb = wpool.tile([128, _NG], f32, tag="bh")

        def x_chunk(t, ks, nk):
            nc.sync.dma_start(
                xres[t][:, ks * _TT:(ks + nk) * _TT].rearrange(
                    "p (k t) -> p k t", k=nk),
                xt.rearrange("(k p) t -> p k t", p=128)[
                    :, ks:ks + nk, t * _TT:(t + 1) * _TT].bitcast(f32r),
            )

        def w_chunk(k):
            nc.sync.dma_start(
                wz_sb[:, k * _EH:(k + 1) * _EH],
                wzt[k * 128:(k + 1) * 128, :].bitcast(f32r))
            nc.sync.dma_start(
                wh_sb[:, k * _EH:(k + 1) * _EH],
                wht[k * 128:(k + 1) * 128, :].bitcast(f32r))

        x_chunk(0, 0, 4)
        w_chunk(0)
        w_chunk(1)
        nc.sync.dma_start(bz_sb[:], bzt)
        nc.sync.dma_start(bh_sb[:], bht)
        x_chunk(0, 4, 4)
        for k in range(2, _NK):
            w_chunk(k)
        for t in range(1, _NT):
            x_chunk(t, 0, 4)
            x_chunk(t, 4, 4)

        def body(first):
          hprev = [None] * _NG
          for t in range(_NT):
            xs = xres[t]
            for g in range(_NG):
                last = (t == _NT - 1 and g == _NG - 1)
                halves = ((0, _TT // 2), (_TT // 2, _TT // 2)) if last \
                    else ((0, _TT),)
                prev_ap = None if t == 0 else hprev[g][:, _TT - 1:_TT]
                for (c0, w) in halves:
                    pool = plast if last else ppool
                    pz = pool.tile([128, w], f32, tag="pzl" if last else "pz")
                    ph = pool.tile([128, w], f32, tag="phl" if last else "ph")
                    for k in range(_NK):
                        nc.tensor.matmul(
                            pz[:],
                            lhsT=wz_sb[:, k * _EH + g * 128:
                                       k * _EH + (g + 1) * 128],
                            rhs=xs[:, k * _TT + c0: k * _TT + c0 + w],
                            start=(k == 0),
                            stop=(k == _NK - 1),
                        )
                    for k in range(_NK):
                        nc.tensor.matmul(
                            ph[:],
                            lhsT=wh_sb[:, k * _EH + g * 128:
                                       k * _EH + (g + 1) * 128],
                            rhs=xs[:, k * _TT + c0: k * _TT + c0 + w],
                            start=(k == 0),
                            stop=(k == _NK - 1),
                        )
                    z = vpool.tile([128, w], f32, tag="z")
                    nc.scalar.activation(z[:], pz[:], AF.Sigmoid,
                                         bias=bz_sb[:, g:g + 1])
                    av = vpool.tile([128, w], f32, tag="a")
                    nc.vector.tensor_scalar(
                        av[:], z[:], -1.0, 1.0, OP.mult, OP.add)
                    bv = vpool.tile([128, w], f32, tag="b")
                    nc.vector.scalar_tensor_tensor(
                        bv[:], ph[:], bh_sb[:, g:g + 1], z[:],
                        OP.add, OP.mult)
                    hb = hpool.tile([128, w], bf16, tag=f"h{g}")
                    init = 0.0 if prev_ap is None else prev_ap
                    nc.vector.tensor_tensor_scan(hb[:], av[:], bv[:], init,
                                                 OP.mult, OP.add)
                    prev_ap = hb[:, w - 1:w]
                    if not last:
                        hprev[g] = hb
                    nc.sync.dma_start(
                        hout[g * 128:(g + 1) * 128,
                             t * _TT + c0: t * _TT + c0 + w], hb[:])

        if loop_n is not None:
            body(True)
            from concourse import mybir as _mb
            with tc.For_i(0, loop_n, 1, hint_engines=(
                    _mb.EngineType.PE, _mb.EngineType.SP,
                    _mb.EngineType.DVE, _mb.EngineType.Activation),
                    staggered_reset=True):
                body(False)
        else:
            for rep in range(reps):
                body(rep == 0)
    nc.compile()
    return nc


_NC_CACHE = None


def _shard_inputs(inputs):
    x = np.asarray(inputs["x"], dtype=np.float32)
    Wz = np.asarray(inputs["Wz"], dtype=np.float32)
    bz = np.asarray(inputs["bz"], dtype=np.float32)
    Wh = np.asarray(inputs["Wh"], dtype=np.float32)
    bh = np.asarray(inputs["bh"], dtype=np.float32)

    wzT = np.ascontiguousarray(Wz.T)
    whT = np.ascontiguousarray(Wh.T)

    in_maps = []
    for i in range(8):
        b, j = i // 2, i % 2
        sl = slice(j * _EH, (j + 1) * _EH)
        in_maps.append({
            "xt": np.ascontiguousarray(x[b].T),
            "wzt": np.ascontiguousarray(wzT[:, sl]),
            "wht": np.ascontiguousarray(whT[:, sl]),
            "bzt": np.ascontiguousarray(bz[sl].reshape(_NG, 128).T),
            "bht": np.ascontiguousarray(bh[sl].reshape(_NG, 128).T),
        })
    return in_maps


def run(inputs, trace=False, tmpdir=None):
    global _NC_CACHE
    from concourse.bass_utils import run_bass_kernel_spmd

    if _NC_CACHE is None:
        _NC_CACHE = _build()
    nc = _NC_CACHE
    in_maps = _shard_inputs(inputs)
    res = run_bass_kernel_spmd(
        nc, in_maps, core_ids=list(range(8)), trace=trace, tmpdir=tmpdir)
    out = np.empty((_B, _T, _D), dtype=np.float32)
    for i in range(8):
        b, j = i // 2, i % 2
        out[b, :, j * _EH:(j + 1) * _EH] = res.results[i]["h"].astype(np.float32).T
    return out, res


def kernel(**inputs):
    out, _ = run(inputs, trace=False)
    return out


# revision 11
# speedup vs baseline: 1.0470x; 1.0156x over previous
"""MinGRU v5 candidate: bf16 matmuls + LDW dedupe + x resident in SBUF +
staggered_reset. Hypothesis: half the SBUF read energy of fp32r buys a
longer full-clock dwell under the power governor."""

import numpy as np

_B, _T, _D = 4, 4096, 1024
_EH = 512
_NG = _EH // 128
_TT = 1024         # timestep supertile (2 PSUM banks)
_NT = _T // _TT    # 4 resident t-supertiles
_NK = _D // 128
_HF = 512


def _dedupe_ldweights(nc):
    from concourse import mybir as mb

    removed = 0
    for fn in nc.m.functions:
        for blk in fn.blocks:
            insts = blk.instructions
            last_sig = None
            dels = []
            for i, inst in enumerate(insts):
                if isinstance(inst, mb.InstLdweights):
                    ap = inst.ins[0]
                    sig = (str(ap.memref), ap.offset, str(ap.ap),
                           str(ap.dtype), inst.perf_mode, inst.tile_position)
                    si = inst.sync_info
                    clean = si is None or (not si.on_wait and not si.on_update)
                    if sig == last_sig and clean:
                        dels.append(i)
                    else:
                        last_sig = sig
                elif isinstance(inst, mb.InstMatmult):
                    if inst.ldweights is not False:
                        last_sig = None
            for i in reversed(dels):
                del insts[i]
            removed += len(dels)
    return removed


def _build(reps=1, loop_n=None):
    from contextlib import ExitStack
    from concourse import bacc, mybir, tile

    f32 = mybir.dt.float32
    bf16 = mybir.dt.bfloat16
    AF = mybir.ActivationFunctionType
    OP = mybir.AluOpType

    nc = bacc.Bacc("TRN2", debug=False, num_devices=8)
    xt = nc.dram_tensor("xt", [_D, _T], bf16, kind="ExternalInput").ap()
    wzt = nc.dram_tensor("wzt", [_D, _EH], bf16, kind="ExternalInput").ap()
    wht = nc.dram_tensor("wht", [_D, _EH], bf16, kind="ExternalInput").ap()
    bzt = nc.dram_tensor("bzt", [128, _NG], f32, kind="ExternalInput").ap()
    bht = nc.dram_tensor("bht", [128, _NG], f32, kind="ExternalInput").ap()
    hout = nc.dram_tensor("h", [_EH, _T], bf16, kind="ExternalOutput").ap()

    with tile.TileContext(nc) as tc, ExitStack() as ctx:
        wpool = ctx.enter_context(tc.tile_pool(name="w", bufs=1))
        vpool = ctx.enter_context(tc.tile_pool(name="v", bufs=3))
        hpool = ctx.enter_context(tc.tile_pool(name="h", bufs=2))
        ppool = ctx.enter_context(tc.tile_pool(name="p", bufs=2, space="PSUM"))

        # x fully resident: 4 supertiles of [128, (k tt)] bf16 = 64 KB/part.
        xres = []
        for t2 in range(_NT):
            xres_t = wpool.tile([128, _NK * _TT], bf16, tag=f"x{t2}")
            xres.append(xres_t)
        wz_sb = wpool.tile([128, _NK * _EH], bf16, tag="wz")
        wh_sb = wpool.tile([128, _NK * _EH], bf16, tag="wh")
        bz_sb = wpool.tile([128, _NG], f32, tag="bz")
        bh_sb = wpool.tile([128, _NG], f32, tag="bh")

        def x_chunk(t2, ks, nk):
            nc.sync.dma_start(
                xres[t2][:, ks * _TT:(ks + nk) * _TT].rearrange(
                    "p (k t) -> p k t", k=nk),
                xt.rearrange("(k p) t -> p k t", p=128)[
                    :, ks:ks + nk, t2 * _TT:(t2 + 1) * _TT],
            )

        def w_chunk(k):
            nc.sync.dma_start(
                wz_sb[:, k * _EH:(k + 1) * _EH],
                wzt[k * 128:(k + 1) * 128, :])
            nc.sync.dma_start(
                wh_sb[:, k * _EH:(k + 1) * _EH],
                wht[k * 128:(k + 1) * 128, :])

        x_chunk(0, 0, 4)
        w_chunk(0)
        w_chunk(1)
        nc.sync.dma_start(bz_sb[:], bzt)
        nc.sync.dma_start(bh_sb[:], bht)
        x_chunk(0, 4, 4)
        for k in range(2, _NK):
            w_chunk(k)
        for t2 in range(1, _NT):
            x_chunk(t2, 0, 4)
            x_chunk(t2, 4, 4)

        def body(first):
          hprev = [None] * _NG
          for t2 in range(_NT):
            xs = xres[t2]
            for g in range(_NG):
                last = (t2 == _NT - 1 and g == _NG - 1)
                pz = ppool.tile([128, _TT], f32, tag="pz")
                ph = ppool.tile([128, _TT], f32, tag="ph")
                for k in range(_NK):
                    for c0 in (0, _HF):
                        nc.tensor.matmul(
                            pz[:, c0:c0 + _HF],
                            lhsT=wz_sb[:, k * _EH + g * 128:
                                       k * _EH + (g + 1) * 128],
                            rhs=xs[:, k * _TT + c0: k * _TT + c0 + _HF],
                            start=(k == 0),
                            stop=(k == _NK - 1),
                        )
                for k in range(_NK):
                    for c0 in (0, _HF):
                        nc.tensor.matmul(
                            ph[:, c0:c0 + _HF],
                            lhsT=wh_sb[:, k * _EH + g * 128:
                                       k * _EH + (g + 1) * 128],
                            rhs=xs[:, k * _TT + c0: k * _TT + c0 + _HF],
                            start=(k == 0),
                            stop=(k == _NK - 1),
                        )
                z = vpool.tile([128, _TT], f32, tag="z")
                av = vpool.tile([128, _TT], f32, tag="a")
                bv = vpool.tile([128, _TT], f32, tag="b")
                hb = hpool.tile([128, _TT], bf16, tag=f"h{g}")
                init = 0.0 if hprev[g] is None \
                    else hprev[g][:, _TT - 1:_TT]
                halves = ((0, _HF), (_HF, _HF)) if last else ((0, _TT),)
                for (c0, w) in halves:
                    sl = slice(c0, c0 + w)
                    nc.scalar.activation(z[:, sl], pz[:, sl], AF.Sigmoid,
                                         bias=bz_sb[:, g:g + 1])
                    nc.vector.tensor_scalar(
                        av[:, sl], z[:, sl], -1.0, 1.0, OP.mult, OP.add)
                    nc.vector.scalar_tensor_tensor(
                        bv[:, sl], ph[:, sl], bh_sb[:, g:g + 1], z[:, sl],
                        OP.add, OP.mult)
                    nc.vector.tensor_tensor_scan(
                        hb[:, sl], av[:, sl], bv[:, sl], init,
                        OP.mult, OP.add)
                    init = hb[:, c0 + w - 1:c0 + w]
                    nc.sync.dma_start(
                        hout[g * 128:(g + 1) * 128,
                             t2 * _TT + c0: t2 * _TT + c0 + w],
                        hb[:, sl])
                hprev[g] = hb

        if loop_n is not None:
            body(True)
            from concourse import mybir as _mb
            with tc.For_i(0, loop_n, 1, hint_engines=(
                    _mb.EngineType.PE, _mb.EngineType.SP,
                    _mb.EngineType.DVE, _mb.EngineType.Activation),
                    staggered_reset=True):
                body(False)
        else:
            for rep in range(reps):
                body(rep == 0)

    _dedupe_ldweights(nc)
    nc.compile()
    return nc


_NC_CACHE = None


def _shard_inputs(inputs):
    import ml_dtypes

    bf16 = ml_dtypes.bfloat16
    x = np.asarray(inputs["x"], dtype=np.float32)
    Wz = np.asarray(inputs["Wz"], dtype=np.float32)
    bz = np.asarray(inputs["bz"], dtype=np.float32)
    Wh = np.asarray(inputs["Wh"], dtype=np.float32)
    bh = np.asarray(inputs["bh"], dtype=np.float32)

    wzT = np.ascontiguousarray(Wz.T).astype(bf16)
    whT = np.ascontiguousarray(Wh.T).astype(bf16)

    in_maps = []
    for i in range(8):
        b, j = i // 2, i % 2
        sl = slice(j * _EH, (j + 1) * _EH)
        in_maps.append({
            "xt": np.ascontiguousarray(x[b].T).astype(bf16),
            "wzt": np.ascontiguousarray(wzT[:, sl]),
            "wht": np.ascontiguousarray(whT[:, sl]),
            "bzt": np.ascontiguousarray(bz[sl].reshape(_NG, 128).T),
            "bht": np.ascontiguousarray(bh[sl].reshape(_NG, 128).T),
        })
    return in_maps


def run(inputs, trace=False, tmpdir=None):
    global _NC_CACHE
    from concourse.bass_utils import run_bass_kernel_spmd

    if _NC_CACHE is None:
        _NC_CACHE = _build()
    nc = _NC_CACHE
    in_maps = _shard_inputs(inputs)
    res = run_bass_kernel_spmd(
        nc, in_maps, core_ids=list(range(8)), trace=trace, tmpdir=tmpdir)
    out = np.empty((_B, _T, _D), dtype=np.float32)
    for i in range(8):
        b, j = i // 2, i % 2
        out[b, :, j * _EH:(j + 1) * _EH] = res.results[i]["h"].astype(np.float32).T
    return out, res


def kernel(**inputs):
    out, _ = run(inputs, trace=False)
    return out


# revision 12
# speedup vs baseline: 1.0532x; 1.0059x over previous
"""MinGRU (B=4, T=4096, D=1024) Trainium2 kernel, 8-core SPMD.

Sharding: core i handles (batch b = i//2, output-channel half j = i%2).
Each core computes u_z = x[b] @ Wz[half].T, u_h = x[b] @ Wh[half].T,
z = sigmoid(u_z + bz), a = 1 - z, bvec = z * (u_h + bh), then the
recurrence h_t = a_t * h_{t-1} + b_t via the hardware tensor_tensor_scan.

Design (all empirically A/B-tested on hardware):
- bf16 matmuls: reads half the SBUF bytes per MAC of fp32r, which holds
  speed under the chip's power-driven clock governor where fp32r drifts
  up (interleaved A/B: bf16 149-152us vs fp32r 159-166us).
- Each 128x128 weight tile streams against two 512-col moving tiles
  into a [128,1024] 2-bank PSUM supertile; _dedupe_ldweights deletes
  the redundant second InstLdweights post-legalization (the in-order PE
  reuses the resident stationary operand - verified bit-correct).
- x fully resident in SBUF (64 KB/partition), loaded once at startup:
  the steady-state loop's only DMA is the bf16 h output.
- One PSUM reader per bank: ACT does z = sigmoid(pz + bz); DVE does
  a = 1 - z, b = (ph + bh) * z, and the scan (fp32 state, bf16 out).
- For_i(staggered_reset=True) removes the per-iteration all-engine
  barrier.
Host pre-transposes x and W so every DMA is row-contiguous and converts
h back to fp32.
"""

import numpy as np

_B, _T, _D = 4, 4096, 1024
_EH = 512
_NG = _EH // 128
_TT = 1024         # timestep supertile (2 PSUM banks)
_NT = _T // _TT    # 4 resident t-supertiles
_NK = _D // 128
_HF = 512


def _dedupe_ldweights(nc):
    from concourse import mybir as mb

    removed = 0
    for fn in nc.m.functions:
        for blk in fn.blocks:
            insts = blk.instructions
            last_sig = None
            dels = []
            for i, inst in enumerate(insts):
                if isinstance(inst, mb.InstLdweights):
                    ap = inst.ins[0]
                    sig = (str(ap.memref), ap.offset, str(ap.ap),
                           str(ap.dtype), inst.perf_mode, inst.tile_position)
                    si = inst.sync_info
                    clean = si is None or (not si.on_wait and not si.on_update)
                    if sig == last_sig and clean:
                        dels.append(i)
                    else:
                        last_sig = sig
                elif isinstance(inst, mb.InstMatmult):
                    if inst.ldweights is not False:
                        last_sig = None
            for i in reversed(dels):
                del insts[i]
            removed += len(dels)
    return removed


def _build(reps=1, loop_n=None):
    from contextlib import ExitStack
    from concourse import bacc, mybir, tile

    f32 = mybir.dt.float32
    bf16 = mybir.dt.bfloat16
    AF = mybir.ActivationFunctionType
    OP = mybir.AluOpType

    nc = bacc.Bacc("TRN2", debug=False, num_devices=8)
    xt = nc.dram_tensor("xt", [_D, _T], bf16, kind="ExternalInput").ap()
    wzt = nc.dram_tensor("wzt", [_D, _EH], bf16, kind="ExternalInput").ap()
    wht = nc.dram_tensor("wht", [_D, _EH], bf16, kind="ExternalInput").ap()
    bzt = nc.dram_tensor("bzt", [128, _NG], f32, kind="ExternalInput").ap()
    bht = nc.dram_tensor("bht", [128, _NG], f32, kind="ExternalInput").ap()
    hout = nc.dram_tensor("h", [_EH, _T], bf16, kind="ExternalOutput").ap()

    with tile.TileContext(nc) as tc, ExitStack() as ctx:
        wpool = ctx.enter_context(tc.tile_pool(name="w", bufs=1))
        vpool = ctx.enter_context(tc.tile_pool(name="v", bufs=3))
        hpool = ctx.enter_context(tc.tile_pool(name="h", bufs=2))
        ppool = ctx.enter_context(tc.tile_pool(name="p", bufs=2, space="PSUM"))

        # x fully resident: 4 supertiles of [128, (k tt)] bf16 = 64 KB/part.
        xres = []
        for t2 in range(_NT):
            xres_t = wpool.tile([128, _NK * _TT], bf16, tag=f"x{t2}")
            xres.append(xres_t)
        wz_sb = wpool.tile([128, _NK * _EH], bf16, tag="wz")
        wh_sb = wpool.tile([128, _NK * _EH], bf16, tag="wh")
        bz_sb = wpool.tile([128, _NG], f32, tag="bz")
        bh_sb = wpool.tile([128, _NG], f32, tag="bh")

        def x_chunk(t2, ks, nk):
            nc.sync.dma_start(
                xres[t2][:, ks * _TT:(ks + nk) * _TT].rearrange(
                    "p (k t) -> p k t", k=nk),
                xt.rearrange("(k p) t -> p k t", p=128)[
                    :, ks:ks + nk, t2 * _TT:(t2 + 1) * _TT],
            )

        def w_chunk(k):
            nc.sync.dma_start(
                wz_sb[:, k * _EH:(k + 1) * _EH],
                wzt[k * 128:(k + 1) * 128, :])
            nc.sync.dma_start(
                wh_sb[:, k * _EH:(k + 1) * _EH],
                wht[k * 128:(k + 1) * 128, :])

        x_chunk(0, 0, 4)
        w_chunk(0)
        w_chunk(1)
        nc.sync.dma_start(bz_sb[:], bzt)
        nc.sync.dma_start(bh_sb[:], bht)
        x_chunk(0, 4, 4)
        for k in range(2, _NK):
            w_chunk(k)
        for t2 in range(1, _NT):
            x_chunk(t2, 0, 4)
            x_chunk(t2, 4, 4)

        def body(first):
          hprev = [None] * _NG
          for t2 in range(_NT):
            xs = xres[t2]
            for g in range(_NG):
                last = (t2 == _NT - 1 and g == _NG - 1)
                pz = ppool.tile([128, _TT], f32, tag="pz")
                ph = ppool.tile([128, _TT], f32, tag="ph")
                for k in range(_NK):
                    for c0 in (0, _HF):
                        nc.tensor.matmul(
                            pz[:, c0:c0 + _HF],
                            lhsT=wz_sb[:, k * _EH + g * 128:
                                       k * _EH + (g + 1) * 128],
                            rhs=xs[:, k * _TT + c0: k * _TT + c0 + _HF],
                            start=(k == 0),
                            stop=(k == _NK - 1),
                        )
                for k in range(_NK):
                    for c0 in (0, _HF):
                        nc.tensor.matmul(
                            ph[:, c0:c0 + _HF],
                            lhsT=wh_sb[:, k * _EH + g * 128:
                                       k * _EH + (g + 1) * 128],
                            rhs=xs[:, k * _TT + c0: k * _TT + c0 + _HF],
                            start=(k == 0),
                            stop=(k == _NK - 1),
                        )
                z = vpool.tile([128, _TT], f32, tag="z")
                av = vpool.tile([128, _TT], f32, tag="a")
                bv = vpool.tile([128, _TT], f32, tag="b")
                hb = hpool.tile([128, _TT], bf16, tag=f"h{g}")
                init = 0.0 if hprev[g] is None \
                    else hprev[g][:, _TT - 1:_TT]
                halves = ((0, _HF), (_HF, _HF)) if last else ((0, _TT),)
                for (c0, w) in halves:
                    sl = slice(c0, c0 + w)
                    nc.scalar.activation(z[:, sl], pz[:, sl], AF.Sigmoid,
                                         bias=bz_sb[:, g:g + 1])
                    nc.vector.tensor_scalar(
                        av[:, sl], z[:, sl], -1.0, 1.0, OP.mult, OP.add)
                    nc.vector.scalar_tensor_tensor(
                        bv[:, sl], ph[:, sl], bh_sb[:, g:g + 1], z[:, sl],
                        OP.add, OP.mult)
                    nc.vector.tensor_tensor_scan(
                        hb[:, sl], av[:, sl], bv[:, sl], init,
                        OP.mult, OP.add)
                    init = hb[:, c0 + w - 1:c0 + w]
                    nc.sync.dma_start(
                        hout[g * 128:(g + 1) * 128,
                             t2 * _TT + c0: t2 * _TT + c0 + w],
                        hb[:, sl])
                hprev[g] = hb

        if loop_n is not None:
            body(True)
            from concourse import mybir as _mb
            with tc.For_i(0, loop_n, 1, hint_engines=(
                    _mb.EngineType.PE, _mb.EngineType.SP,
                    _mb.EngineType.DVE, _mb.EngineType.Activation),
                    staggered_reset=True):
                body(False)
        else:
            for rep in range(reps):
                body(rep == 0)

    _dedupe_ldweights(nc)
    nc.compile()
    return nc


_NC_CACHE = None


def _shard_inputs(inputs):
    import ml_dtypes

    bf16 = ml_dtypes.bfloat16
    x = np.asarray(inputs["x"], dtype=np.float32)
    Wz = np.asarray(inputs["Wz"], dtype=np.float32)
    bz = np.asarray(inputs["bz"], dtype=np.float32)
    Wh = np.asarray(inputs["Wh"], dtype=np.float32)
    bh = np.asarray(inputs["bh"], dtype=np.float32)

    wzT = np.ascontiguousarray(Wz.T).astype(bf16)
    whT = np.ascontiguousarray(Wh.T).astype(bf16)

    in_maps = []
    for i in range(8):
        b, j = i // 2, i % 2
        sl = slice(j * _EH, (j + 1) * _EH)
        in_maps.append({
            "xt": np.ascontiguousarray(x[b].T).astype(bf16),
            "wzt": np.ascontiguousarray(wzT[:, sl]),
            "wht": np.ascontiguousarray(whT[:, sl]),
            "bzt": np.ascontiguousarray(bz[sl].reshape(_NG, 128).T),
            "bht": np.ascontiguousarray(bh[sl].reshape(_NG, 128).T),
        })
    return in_maps


def run(inputs, trace=False, tmpdir=None):
    global _NC_CACHE
    from concourse.bass_utils import run_bass_kernel_spmd

    if _NC_CACHE is None:
        _NC_CACHE = _build()
    nc = _NC_CACHE
    in_maps = _shard_inputs(inputs)
    res = run_bass_kernel_spmd(
        nc, in_maps, core_ids=list(range(8)), trace=trace, tmpdir=tmpdir)
    out = np.empty((_B, _T, _D), dtype=np.float32)
    for i in range(8):
        b, j = i // 2, i % 2
        out[b, :, j * _EH:(j + 1) * _EH] = res.results[i]["h"].astype(np.float32).T
    return out, res


def kernel(**inputs):
    out, _ = run(inputs, trace=False)
    return out
